# revision 2
# baseline (speedup 1.0000x reference)
"""KV-cache ring-buffer update + rolling re-linearization, on 8 trn2 NeuronCores.

Problem semantics (nn_KVCache):
  k_cache[:, pos] = k ; v_cache[:, pos] = v          (scatter into ring buffer)
  out = concat([cache[:, split:], cache[:, :split]]) (roll to logical order)

For the given inputs (pos = arange(7680..8703) % 8192, max_pos = 8703) the
whole computation reduces to contiguous row copies:
  out[:, 0:7168]    = cache[:, 512:7680]   (old data, 28 MiB per tensor/batch)
  out[:, 7168:8192] = new[:, 0:1024]       ( 4 MiB per tensor/batch)

Sharding: pure batch-parallel (B=8 -> 1 batch per core), no communication.
Each core runs DRAM->DRAM HWDGE DMA copies on both hardware DGE rings.

Perf notes (hardware-trace driven):
- The copy is aggregate-HBM-bound: ~330 GB/s copy (read+write ~660 GB/s)
  per core across the 16 SDMA engines; per-engine sustained ~21 GB/s.
- SDMA engine 15 stochastically runs ~17% slower (roughly 1/3 of runs,
  likely external RMTV/AXI contention) and can straggle the whole kernel
  by +35..95 us. Every dma_start sprays uniformly over its ring's engine
  slots (engine choice is NOT addressable), but DMAQueue.num_queues=N
  confines a ring to engine slots 0..N-1.
- So: the sync/SP ring keeps 16 slots and carries k (32 MiB) plus the head
  of v (18 MiB); the scalar/Act ring is declared with 15 slots (engine 15
  excluded) and carries the 14 MiB tail of v. Engine 15 then owns only
  50/16 = 3.1 MiB: even at its pathological rate it finishes inside the
  window, while healthy runs keep it usefully busy.
- A 15-slot ring stripes unevenly (engines 0-7 get the largest share),
  which is why the Act ring's load is kept small.
- Each dma_start's sem descs add value/16 per participating engine, so an
  instruction on the 15-slot ring increments its semaphore by 15.
- Final v/k instructions use 16 KiB descriptors to shrink the finish
  quantum.
"""

import numpy as np

B, S_NEW, H, D = 8, 1024, 16, 128
MAX_SIZE = 8192
HD = H * D  # 2048 fp16 elements = 4096 B per row

N_CORES = 8

# rows the scalar/Act ring carries (tail of the v tensor), 14 MiB
ACT_ROWS = 3584


def _copy_plan(pos, max_pos):
    """Derive the list of contiguous row-copies implied by (pos, max_pos).

    Returns (out_rows, runs) with runs = [(dst_row, src: 'new'|'cache',
    src_row, n_rows), ...] such that
      out[dst:dst+n] = (k|v)[src_row:src_row+n]        if src == 'new'
      out[dst:dst+n] = (k|v)_cache[src_row:src_row+n]  if src == 'cache'
    """
    pos = (np.asarray(pos).astype(np.int64) % MAX_SIZE).ravel()
    next_pos = int(max_pos) + 1
    if next_pos > MAX_SIZE:
        out_rows = MAX_SIZE
        split = next_pos % MAX_SIZE
        order = (np.arange(MAX_SIZE, dtype=np.int64) + split) % MAX_SIZE
    else:
        out_rows = next_pos
        order = np.arange(next_pos, dtype=np.int64)
    newpos = np.full(MAX_SIZE, -1, dtype=np.int64)
    newpos[pos] = np.arange(pos.shape[0], dtype=np.int64)  # duplicate pos: last wins
    sel = newpos[order]
    is_new = sel >= 0
    src_row = np.where(is_new, sel, order)
    runs = []
    j = 0
    while j < out_rows:
        s = j
        while (
            j + 1 < out_rows
            and is_new[j + 1] == is_new[s]
            and src_row[j + 1] == src_row[s] + (j + 1 - s)
        ):
            j += 1
        runs.append((s, "new" if is_new[s] else "cache", int(src_row[s]), j - s + 1))
        j += 1
    return out_rows, runs


_EXPECTED_RUNS = [(0, "cache", 512, 7168), (7168, "new", 0, 1024)]


def _build_fast(out_rows):
    """Tuned build for the expected copy plan (see module docstring)."""
    import concourse.bass as bass
    import concourse.mybir as mybir

    nc = bass.Bass()
    f16 = mybir.dt.float16
    kc = nc.declare_dram_parameter("kc", [MAX_SIZE, HD], f16, isOutput=False)
    vc = nc.declare_dram_parameter("vc", [MAX_SIZE, HD], f16, isOutput=False)
    kn = nc.declare_dram_parameter("kn", [S_NEW, HD], f16, isOutput=False)
    vn = nc.declare_dram_parameter("vn", [S_NEW, HD], f16, isOutput=False)
    ko = nc.declare_dram_parameter("ko", [out_rows, HD], f16, isOutput=True)
    vo = nc.declare_dram_parameter("vo", [out_rows, HD], f16, isOutput=True)

    for q in nc.m.queues:
        if q.name == "qActDynamicHW":
            q.num_queues = 15  # keep engine 15 off the scalar ring

    OLD_DST, OLD_SRC, OLD_N = _EXPECTED_RUNS[0][0], _EXPECTED_RUNS[0][2], _EXPECTED_RUNS[0][3]
    NEW_DST = _EXPECTED_RUNS[1][0]
    v_split = out_rows - ACT_ROWS

    with (
        nc.Block(no_gpsimd_drain=True) as block,
        nc.semaphore("k_sem") as k_sem,
        nc.semaphore("v_sem") as v_sem,
    ):
        @block.sync
        def _(sync):
            sync.dma_start(out=ko[OLD_DST:OLD_DST + OLD_N],
                           in_=kc[OLD_SRC:OLD_SRC + OLD_N]).then_inc(k_sem, 16)
            sync.dma_start(out=ko[NEW_DST:NEW_DST + 768],
                           in_=kn[0:768]).then_inc(k_sem, 16)
            sync.dma_start(out=ko[NEW_DST + 768:NEW_DST + 1024],
                           in_=kn[768:1024],
                           max_dma_last_dim=16384).then_inc(k_sem, 16)
            sync.dma_start(out=vo[0:v_split],
                           in_=vc[OLD_SRC:OLD_SRC + v_split]).then_inc(k_sem, 16)
            sync.wait_ge(k_sem, 16 * 4)

        @block.scalar
        def _(scalar):
            n_old_tail = OLD_N - v_split
            scalar.dma_start(out=vo[v_split:v_split + n_old_tail],
                             in_=vc[OLD_SRC + v_split:OLD_SRC + OLD_N]).then_inc(v_sem, 16)
            scalar.dma_start(out=vo[NEW_DST:NEW_DST + 768],
                             in_=vn[0:768]).then_inc(v_sem, 16)
            scalar.dma_start(out=vo[NEW_DST + 768:NEW_DST + 1024],
                             in_=vn[768:1024],
                             max_dma_last_dim=16384).then_inc(v_sem, 16)
            scalar.wait_ge(v_sem, 15 * 3)

    return nc


def _build_generic(out_rows, runs):
    """Fallback for unexpected (pos, max_pos): k on sync ring, v on scalar."""
    import concourse.bass as bass
    import concourse.mybir as mybir

    nc = bass.Bass()
    f16 = mybir.dt.float16
    kc = nc.declare_dram_parameter("kc", [MAX_SIZE, HD], f16, isOutput=False)
    vc = nc.declare_dram_parameter("vc", [MAX_SIZE, HD], f16, isOutput=False)
    kn = nc.declare_dram_parameter("kn", [S_NEW, HD], f16, isOutput=False)
    vn = nc.declare_dram_parameter("vn", [S_NEW, HD], f16, isOutput=False)
    ko = nc.declare_dram_parameter("ko", [out_rows, HD], f16, isOutput=True)
    vo = nc.declare_dram_parameter("vo", [out_rows, HD], f16, isOutput=True)

    with (
        nc.Block(no_gpsimd_drain=True) as block,
        nc.semaphore("k_sem") as k_sem,
        nc.semaphore("v_sem") as v_sem,
    ):
        @block.sync
        def _(sync):
            for dst, src, row, n in runs:
                sk = kn if src == "new" else kc
                sync.dma_start(out=ko[dst:dst + n], in_=sk[row:row + n]).then_inc(k_sem, 16)
            sync.wait_ge(k_sem, 16 * len(runs))

        @block.scalar
        def _(scalar):
            for dst, src, row, n in runs:
                sv = vn if src == "new" else vc
                scalar.dma_start(out=vo[dst:dst + n], in_=sv[row:row + n]).then_inc(v_sem, 16)
            scalar.wait_ge(v_sem, 16 * len(runs))

    return nc


def _run(k, v, k_cache, v_cache, pos, max_pos, trace=False):
    from concourse.bass_utils import run_bass_kernel_spmd

    k = np.asarray(k)
    v = np.asarray(v)
    k_cache = np.asarray(k_cache)
    v_cache = np.asarray(v_cache)

    out_rows, runs = _copy_plan(pos, max_pos)
    if runs == _EXPECTED_RUNS and out_rows == MAX_SIZE:
        nc = _build_fast(out_rows)
    else:
        nc = _build_generic(out_rows, runs)

    in_maps = [
        {
            "kc": k_cache[b].reshape(MAX_SIZE, HD),
            "vc": v_cache[b].reshape(MAX_SIZE, HD),
            "kn": k[b].reshape(S_NEW, HD),
            "vn": v[b].reshape(S_NEW, HD),
        }
        for b in range(N_CORES)
    ]
    res = run_bass_kernel_spmd(nc, in_maps, list(range(N_CORES)), trace=trace)
    k_out = np.stack([r["ko"] for r in res.results]).reshape(B, out_rows, H, D)
    v_out = np.stack([r["vo"] for r in res.results]).reshape(B, out_rows, H, D)
    return (k_out, v_out), res


def kernel(k, v, k_cache, v_cache, pos, max_pos):
    (k_out, v_out), _ = _run(k, v, k_cache, v_cache, pos, max_pos)
    return k_out, v_out


# revision 4
# speedup vs baseline: 1.0092x; 1.0092x over previous
"""KV-cache ring-buffer update + rolling re-linearization, on 8 trn2 NeuronCores.

Problem semantics (nn_KVCache):
  k_cache[:, pos] = k ; v_cache[:, pos] = v          (scatter into ring buffer)
  out = concat([cache[:, split:], cache[:, :split]]) (roll to logical order)

For the given inputs (pos = arange(7680..8703) % 8192, max_pos = 8703) the
whole computation reduces to contiguous row copies:
  out[:, 0:7168]    = cache[:, 512:7680]   (old data, 28 MiB per tensor/batch)
  out[:, 7168:8192] = new[:, 0:1024]       ( 4 MiB per tensor/batch)

Sharding: pure batch-parallel (B=8 -> 1 batch per core), no communication.
Each core runs DRAM->DRAM HWDGE DMA copies on both hardware DGE rings.

Perf notes (hardware-trace driven):
- The copy is aggregate-HBM-bound: ~330 GB/s copy (read+write ~660 GB/s)
  per core across the 16 SDMA engines; per-engine sustained ~21 GB/s.
- SDMA engine 15 stochastically runs ~17% slower (roughly 1/3 of runs,
  likely external RMTV/AXI contention) and can straggle the whole kernel
  by +35..95 us. Every dma_start sprays uniformly over its ring's engine
  slots (engine choice is NOT addressable), but DMAQueue.num_queues=N
  confines a ring to engine slots 0..N-1.
- So: the sync/SP ring keeps 16 slots and carries k (32 MiB) plus the head
  of v (18 MiB); the scalar/Act ring is declared with 15 slots (engine 15
  excluded) and carries the 14 MiB tail of v. Engine 15 then owns only
  50/16 = 3.1 MiB: even at its pathological rate it finishes inside the
  window, while healthy runs keep it usefully busy.
- A 15-slot ring stripes unevenly (engines 0-7 get the largest share),
  which is why the Act ring's load is kept small.
- Each dma_start's sem descs add value/16 per participating engine, so an
  instruction on the 15-slot ring increments its semaphore by 15.
- Final v/k instructions use 16 KiB descriptors to shrink the finish
  quantum.
"""

import numpy as np

B, S_NEW, H, D = 8, 1024, 16, 128
MAX_SIZE = 8192
HD = H * D  # 2048 fp16 elements = 4096 B per row

N_CORES = 8

# v out-rows [V_ACT0, 7168) go to the scalar/Act ring (10 MiB, one
# 160-descriptor instruction - validated to stripe as engines 0-9 x 1 MiB)
V_ACT0 = 4608


def _copy_plan(pos, max_pos):
    """Derive the list of contiguous row-copies implied by (pos, max_pos).

    Returns (out_rows, runs) with runs = [(dst_row, src: 'new'|'cache',
    src_row, n_rows), ...] such that
      out[dst:dst+n] = (k|v)[src_row:src_row+n]        if src == 'new'
      out[dst:dst+n] = (k|v)_cache[src_row:src_row+n]  if src == 'cache'
    """
    pos = (np.asarray(pos).astype(np.int64) % MAX_SIZE).ravel()
    next_pos = int(max_pos) + 1
    if next_pos > MAX_SIZE:
        out_rows = MAX_SIZE
        split = next_pos % MAX_SIZE
        order = (np.arange(MAX_SIZE, dtype=np.int64) + split) % MAX_SIZE
    else:
        out_rows = next_pos
        order = np.arange(next_pos, dtype=np.int64)
    newpos = np.full(MAX_SIZE, -1, dtype=np.int64)
    newpos[pos] = np.arange(pos.shape[0], dtype=np.int64)  # duplicate pos: last wins
    sel = newpos[order]
    is_new = sel >= 0
    src_row = np.where(is_new, sel, order)
    runs = []
    j = 0
    while j < out_rows:
        s = j
        while (
            j + 1 < out_rows
            and is_new[j + 1] == is_new[s]
            and src_row[j + 1] == src_row[s] + (j + 1 - s)
        ):
            j += 1
        runs.append((s, "new" if is_new[s] else "cache", int(src_row[s]), j - s + 1))
        j += 1
    return out_rows, runs


_EXPECTED_RUNS = [(0, "cache", 512, 7168), (7168, "new", 0, 1024)]


def _build_fast(out_rows):
    """Tuned build for the expected copy plan (see module docstring)."""
    import concourse.bass as bass
    import concourse.mybir as mybir

    nc = bass.Bass()
    f16 = mybir.dt.float16
    kc = nc.declare_dram_parameter("kc", [MAX_SIZE, HD], f16, isOutput=False)
    vc = nc.declare_dram_parameter("vc", [MAX_SIZE, HD], f16, isOutput=False)
    kn = nc.declare_dram_parameter("kn", [S_NEW, HD], f16, isOutput=False)
    vn = nc.declare_dram_parameter("vn", [S_NEW, HD], f16, isOutput=False)
    ko = nc.declare_dram_parameter("ko", [out_rows, HD], f16, isOutput=True)
    vo = nc.declare_dram_parameter("vo", [out_rows, HD], f16, isOutput=True)

    for q in nc.m.queues:
        if q.name == "qActDynamicHW":
            q.num_queues = 15  # keep engine 15 off the scalar ring

    OLD_DST, OLD_SRC, OLD_N = _EXPECTED_RUNS[0][0], _EXPECTED_RUNS[0][2], _EXPECTED_RUNS[0][3]
    NEW_DST = _EXPECTED_RUNS[1][0]

    with (
        nc.Block(no_gpsimd_drain=True) as block,
        nc.semaphore("k_sem") as k_sem,
        nc.semaphore("v_sem") as v_sem,
    ):
        @block.sync
        def _(sync):
            # 54 MiB, all desc counts %16==0: uniform 3.375 MiB per engine
            sync.dma_start(out=ko[OLD_DST:OLD_DST + OLD_N],
                           in_=kc[OLD_SRC:OLD_SRC + OLD_N]).then_inc(k_sem, 16)
            sync.dma_start(out=ko[NEW_DST:NEW_DST + 768],
                           in_=kn[0:768]).then_inc(k_sem, 16)
            sync.dma_start(out=ko[NEW_DST + 768:NEW_DST + 1024],
                           in_=kn[768:1024],
                           max_dma_last_dim=16384).then_inc(k_sem, 16)
            sync.dma_start(out=vo[0:V_ACT0],
                           in_=vc[OLD_SRC:OLD_SRC + V_ACT0]).then_inc(k_sem, 16)
            sync.dma_start(out=vo[NEW_DST:NEW_DST + 768],
                           in_=vn[0:768]).then_inc(k_sem, 16)
            sync.dma_start(out=vo[NEW_DST + 768:NEW_DST + 1024],
                           in_=vn[768:1024],
                           max_dma_last_dim=16384).then_inc(k_sem, 16)
            sync.wait_ge(k_sem, 16 * 6)

        @block.scalar
        def _(scalar):
            # one 160-desc instruction: engines 0-9 get 1 MiB each
            scalar.dma_start(out=vo[V_ACT0:OLD_N],
                             in_=vc[OLD_SRC + V_ACT0:OLD_SRC + OLD_N]).then_inc(v_sem, 16)
            scalar.wait_ge(v_sem, 15)

    return nc


def _build_generic(out_rows, runs):
    """Fallback for unexpected (pos, max_pos): k on sync ring, v on scalar."""
    import concourse.bass as bass
    import concourse.mybir as mybir

    nc = bass.Bass()
    f16 = mybir.dt.float16
    kc = nc.declare_dram_parameter("kc", [MAX_SIZE, HD], f16, isOutput=False)
    vc = nc.declare_dram_parameter("vc", [MAX_SIZE, HD], f16, isOutput=False)
    kn = nc.declare_dram_parameter("kn", [S_NEW, HD], f16, isOutput=False)
    vn = nc.declare_dram_parameter("vn", [S_NEW, HD], f16, isOutput=False)
    ko = nc.declare_dram_parameter("ko", [out_rows, HD], f16, isOutput=True)
    vo = nc.declare_dram_parameter("vo", [out_rows, HD], f16, isOutput=True)

    with (
        nc.Block(no_gpsimd_drain=True) as block,
        nc.semaphore("k_sem") as k_sem,
        nc.semaphore("v_sem") as v_sem,
    ):
        @block.sync
        def _(sync):
            for dst, src, row, n in runs:
                sk = kn if src == "new" else kc
                sync.dma_start(out=ko[dst:dst + n], in_=sk[row:row + n]).then_inc(k_sem, 16)
            sync.wait_ge(k_sem, 16 * len(runs))

        @block.scalar
        def _(scalar):
            for dst, src, row, n in runs:
                sv = vn if src == "new" else vc
                scalar.dma_start(out=vo[dst:dst + n], in_=sv[row:row + n]).then_inc(v_sem, 16)
            scalar.wait_ge(v_sem, 16 * len(runs))

    return nc


def _run(k, v, k_cache, v_cache, pos, max_pos, trace=False):
    from concourse.bass_utils import run_bass_kernel_spmd

    k = np.asarray(k)
    v = np.asarray(v)
    k_cache = np.asarray(k_cache)
    v_cache = np.asarray(v_cache)

    out_rows, runs = _copy_plan(pos, max_pos)
    if runs == _EXPECTED_RUNS and out_rows == MAX_SIZE:
        nc = _build_fast(out_rows)
    else:
        nc = _build_generic(out_rows, runs)

    in_maps = [
        {
            "kc": k_cache[b].reshape(MAX_SIZE, HD),
            "vc": v_cache[b].reshape(MAX_SIZE, HD),
            "kn": k[b].reshape(S_NEW, HD),
            "vn": v[b].reshape(S_NEW, HD),
        }
        for b in range(N_CORES)
    ]
    res = run_bass_kernel_spmd(nc, in_maps, list(range(N_CORES)), trace=trace)
    k_out = np.stack([r["ko"] for r in res.results]).reshape(B, out_rows, H, D)
    v_out = np.stack([r["vo"] for r in res.results]).reshape(B, out_rows, H, D)
    return (k_out, v_out), res


def kernel(k, v, k_cache, v_cache, pos, max_pos):
    (k_out, v_out), _ = _run(k, v, k_cache, v_cache, pos, max_pos)
    return k_out, v_out


# revision 8
# speedup vs baseline: 1.0438x; 1.0343x over previous
"""KV-cache ring-buffer update + rolling re-linearization, on 8 trn2 NeuronCores.

Problem semantics (nn_KVCache):
  k_cache[:, pos] = k ; v_cache[:, pos] = v          (scatter into ring buffer)
  out = concat([cache[:, split:], cache[:, :split]]) (roll to logical order)

For the given inputs (pos = arange(7680..8703) % 8192, max_pos = 8703) the
whole computation reduces to contiguous row copies:
  out[:, 0:7168]    = cache[:, 512:7680]   (old data, 28 MiB per tensor/batch)
  out[:, 7168:8192] = new[:, 0:1024]       ( 4 MiB per tensor/batch)

Sharding: pure batch-parallel (B=8 -> 1 batch per core), no communication.
Each core runs DRAM->DRAM HWDGE DMA copies on both hardware DGE rings.

Perf notes (hardware-trace driven):
- The copy is aggregate-HBM-bound: ~330 GB/s copy (read+write ~660 GB/s)
  per core across the 16 SDMA engines; per-engine sustained ~21 GB/s.
- SDMA engine 15 stochastically runs ~17% slower (roughly 1/3 of runs,
  likely external RMTV/AXI contention) and can straggle the whole kernel
  by +35..95 us. Every dma_start sprays uniformly over its ring's engine
  slots (engine choice is NOT addressable), but DMAQueue.num_queues=N
  confines a ring to engine slots 0..N-1.
- So: the sync/SP ring keeps 16 slots and carries k (32 MiB) plus 17 MiB
  of v, all with descriptor counts % 16 == 0 (uniform 3.0625 MiB/engine);
  the scalar/Act ring is declared with 15 slots (engine 15 excluded) and
  carries one 240-descriptor (15 MiB) v instruction, which the DGE stripes
  perfectly as engines 0-14 x 1 MiB (chunk = next_pow2(ceil(D/slots)) = 16,
  no partial engine). Net: engines 0-14 carry 4.0625 MiB, engine 15 only
  3.0625 MiB - it stays inside the window even at a 12 GB/s pathological
  rate, while healthy runs keep it usefully busy.
- Non-power-of-2 slot counts stripe unevenly for most instruction sizes
  (low engines get the largest share); only chunk-exact sizes like 240
  descs avoid that, which is why the Act ring carries exactly one such
  instruction.
- Each dma_start's sem descs add value/16 per participating engine, so an
  instruction on the 15-slot ring increments its semaphore by 15.
- Final v/k instructions use 16 KiB descriptors to shrink the finish
  quantum.
"""

import numpy as np

B, S_NEW, H, D = 8, 1024, 16, 128
MAX_SIZE = 8192
HD = H * D  # 2048 fp16 elements = 4096 B per row

N_CORES = 8

# v out-rows [V_ACT0, 7168) go to the scalar/Act ring: 3840 rows = 240
# descriptors. ceil(240/15) = 16 is a power of two with no partial engine,
# so the hardware DGE stripes it perfectly uniformly: engines 0-14 x 1 MiB
# (verified by trace: exactly 16 slices per engine, none on engine 15).
V_ACT0 = 3328


def _copy_plan(pos, max_pos):
    """Derive the list of contiguous row-copies implied by (pos, max_pos).

    Returns (out_rows, runs) with runs = [(dst_row, src: 'new'|'cache',
    src_row, n_rows), ...] such that
      out[dst:dst+n] = (k|v)[src_row:src_row+n]        if src == 'new'
      out[dst:dst+n] = (k|v)_cache[src_row:src_row+n]  if src == 'cache'
    """
    pos = (np.asarray(pos).astype(np.int64) % MAX_SIZE).ravel()
    next_pos = int(max_pos) + 1
    if next_pos > MAX_SIZE:
        out_rows = MAX_SIZE
        split = next_pos % MAX_SIZE
        order = (np.arange(MAX_SIZE, dtype=np.int64) + split) % MAX_SIZE
    else:
        out_rows = next_pos
        order = np.arange(next_pos, dtype=np.int64)
    newpos = np.full(MAX_SIZE, -1, dtype=np.int64)
    newpos[pos] = np.arange(pos.shape[0], dtype=np.int64)  # duplicate pos: last wins
    sel = newpos[order]
    is_new = sel >= 0
    src_row = np.where(is_new, sel, order)
    runs = []
    j = 0
    while j < out_rows:
        s = j
        while (
            j + 1 < out_rows
            and is_new[j + 1] == is_new[s]
            and src_row[j + 1] == src_row[s] + (j + 1 - s)
        ):
            j += 1
        runs.append((s, "new" if is_new[s] else "cache", int(src_row[s]), j - s + 1))
        j += 1
    return out_rows, runs


_EXPECTED_RUNS = [(0, "cache", 512, 7168), (7168, "new", 0, 1024)]


def _build_fast(out_rows):
    """Tuned build for the expected copy plan (see module docstring)."""
    import concourse.bass as bass
    import concourse.mybir as mybir

    nc = bass.Bass()
    f16 = mybir.dt.float16
    kc = nc.declare_dram_parameter("kc", [MAX_SIZE, HD], f16, isOutput=False)
    vc = nc.declare_dram_parameter("vc", [MAX_SIZE, HD], f16, isOutput=False)
    kn = nc.declare_dram_parameter("kn", [S_NEW, HD], f16, isOutput=False)
    vn = nc.declare_dram_parameter("vn", [S_NEW, HD], f16, isOutput=False)
    ko = nc.declare_dram_parameter("ko", [out_rows, HD], f16, isOutput=True)
    vo = nc.declare_dram_parameter("vo", [out_rows, HD], f16, isOutput=True)

    for q in nc.m.queues:
        if q.name == "qActDynamicHW":
            q.num_queues = 15  # keep engine 15 off the scalar ring

    OLD_DST, OLD_SRC, OLD_N = _EXPECTED_RUNS[0][0], _EXPECTED_RUNS[0][2], _EXPECTED_RUNS[0][3]
    NEW_DST = _EXPECTED_RUNS[1][0]

    with (
        nc.Block(no_gpsimd_drain=True) as block,
        nc.semaphore("k_sem") as k_sem,
        nc.semaphore("v_sem") as v_sem,
    ):
        @block.sync
        def _(sync):
            # 49 MiB, all desc counts %16==0: uniform 3.0625 MiB per engine
            sync.dma_start(out=ko[OLD_DST:OLD_DST + OLD_N],
                           in_=kc[OLD_SRC:OLD_SRC + OLD_N]).then_inc(k_sem, 16)
            sync.dma_start(out=ko[NEW_DST:NEW_DST + 768],
                           in_=kn[0:768]).then_inc(k_sem, 16)
            sync.dma_start(out=ko[NEW_DST + 768:NEW_DST + 1024],
                           in_=kn[768:1024],
                           max_dma_last_dim=16384).then_inc(k_sem, 16)
            sync.dma_start(out=vo[0:V_ACT0],
                           in_=vc[OLD_SRC:OLD_SRC + V_ACT0]).then_inc(k_sem, 16)
            sync.dma_start(out=vo[NEW_DST:NEW_DST + 768],
                           in_=vn[0:768]).then_inc(k_sem, 16)
            sync.dma_start(out=vo[NEW_DST + 768:NEW_DST + 1024],
                           in_=vn[768:1024],
                           max_dma_last_dim=16384).then_inc(k_sem, 16)
            sync.wait_ge(k_sem, 16 * 6)

        @block.scalar
        def _(scalar):
            # one 240-desc instruction: engines 0-14 get 1 MiB each
            scalar.dma_start(out=vo[V_ACT0:OLD_N],
                             in_=vc[OLD_SRC + V_ACT0:OLD_SRC + OLD_N]).then_inc(v_sem, 16)
            scalar.wait_ge(v_sem, 15)

    return nc


def _build_generic(out_rows, runs):
    """Fallback for unexpected (pos, max_pos): k on sync ring, v on scalar."""
    import concourse.bass as bass
    import concourse.mybir as mybir

    nc = bass.Bass()
    f16 = mybir.dt.float16
    kc = nc.declare_dram_parameter("kc", [MAX_SIZE, HD], f16, isOutput=False)
    vc = nc.declare_dram_parameter("vc", [MAX_SIZE, HD], f16, isOutput=False)
    kn = nc.declare_dram_parameter("kn", [S_NEW, HD], f16, isOutput=False)
    vn = nc.declare_dram_parameter("vn", [S_NEW, HD], f16, isOutput=False)
    ko = nc.declare_dram_parameter("ko", [out_rows, HD], f16, isOutput=True)
    vo = nc.declare_dram_parameter("vo", [out_rows, HD], f16, isOutput=True)

    with (
        nc.Block(no_gpsimd_drain=True) as block,
        nc.semaphore("k_sem") as k_sem,
        nc.semaphore("v_sem") as v_sem,
    ):
        @block.sync
        def _(sync):
            for dst, src, row, n in runs:
                sk = kn if src == "new" else kc
                sync.dma_start(out=ko[dst:dst + n], in_=sk[row:row + n]).then_inc(k_sem, 16)
            sync.wait_ge(k_sem, 16 * len(runs))

        @block.scalar
        def _(scalar):
            for dst, src, row, n in runs:
                sv = vn if src == "new" else vc
                scalar.dma_start(out=vo[dst:dst + n], in_=sv[row:row + n]).then_inc(v_sem, 16)
            scalar.wait_ge(v_sem, 16 * len(runs))

    return nc


def _run(k, v, k_cache, v_cache, pos, max_pos, trace=False):
    from concourse.bass_utils import run_bass_kernel_spmd

    k = np.asarray(k)
    v = np.asarray(v)
    k_cache = np.asarray(k_cache)
    v_cache = np.asarray(v_cache)

    out_rows, runs = _copy_plan(pos, max_pos)
    if runs == _EXPECTED_RUNS and out_rows == MAX_SIZE:
        nc = _build_fast(out_rows)
    else:
        nc = _build_generic(out_rows, runs)

    in_maps = [
        {
            "kc": k_cache[b].reshape(MAX_SIZE, HD),
            "vc": v_cache[b].reshape(MAX_SIZE, HD),
            "kn": k[b].reshape(S_NEW, HD),
            "vn": v[b].reshape(S_NEW, HD),
        }
        for b in range(N_CORES)
    ]
    res = run_bass_kernel_spmd(nc, in_maps, list(range(N_CORES)), trace=trace)
    k_out = np.stack([r["ko"] for r in res.results]).reshape(B, out_rows, H, D)
    v_out = np.stack([r["vo"] for r in res.results]).reshape(B, out_rows, H, D)
    return (k_out, v_out), res


def kernel(k, v, k_cache, v_cache, pos, max_pos):
    (k_out, v_out), _ = _run(k, v, k_cache, v_cache, pos, max_pos)
    return k_out, v_out
